# revision 1
# baseline (speedup 1.0000x reference)
import numpy as np
import concourse.bass as bass
import concourse.bacc as bacc
import concourse.mybir as mybir
from concourse.tile import TileContext
from concourse.bass_utils import run_bass_kernel_spmd

B, HID = 4096, 512
NR, NB = 32, 8
T = 32
OPB, AB, LB, NOPS = 2, 5, 5, 4
G = 8
NCORES = 8
BC = B // NCORES          # 512 batch rows per core
P = 128
NBLK = BC // P            # 4 blocks per core
COLS = NR * NB + T * OPB + 3 * T * AB + LB   # 805

# column offsets inside the concatenated weight matrix
OFF_R, OFF_OP, OFF_D, OFF_1, OFF_2, OFF_L = 0, 256, 320, 480, 640, 800

f32 = mybir.dt.float32
AX = mybir.AxisListType
OP = mybir.AluOpType
AF = mybir.ActivationFunctionType

_STATE = {}


def _build():
    nc = bacc.Bacc("TRN2", target_bir_lowering=False, debug=False,
                   num_devices=NCORES)
    z_d = nc.declare_dram_parameter("z", [BC, HID], f32, isOutput=False)
    wc_d = nc.declare_dram_parameter("wcat", [HID, COLS], f32, isOutput=False)
    pw_d = nc.declare_dram_parameter("pw", [P, COLS], f32, isOutput=False)
    ri_d = nc.declare_dram_parameter("ri", [P, NR], f32, isOutput=False)
    ki_d = nc.declare_dram_parameter("ki", [P, NOPS], f32, isOutput=False)
    tg_d = nc.declare_dram_parameter("tg", [P, T], f32, isOutput=False)
    id_d = nc.declare_dram_parameter("ident", [P, P], f32, isOutput=False)
    w2_d = nc.declare_dram_parameter("w2tb", [NR + 1, HID], f32, isOutput=False)
    lg_d = nc.declare_dram_parameter("lng", [P, HID], f32, isOutput=False)
    lb_d = nc.declare_dram_parameter("lnb", [P, HID], f32, isOutput=False)
    out_d = nc.declare_dram_parameter("out", [BC, G * HID], f32, isOutput=True)

    delta = np.linspace(-1.0, 1.0, G).astype(np.float32)

    with TileContext(nc) as tc:
        with tc.tile_pool(name="const", bufs=1) as cp, \
             tc.tile_pool(name="work", bufs=1) as wp, \
             tc.tile_pool(name="scr", bufs=2) as sp, \
             tc.psum_pool(name="pst", bufs=2) as pt, \
             tc.psum_pool(name="psl", bufs=1) as pl, \
             tc.psum_pool(name="psh", bufs=2) as ph:
            # ---- constants ----
            wc = cp.tile([P, 4, COLS], f32)
            nc.gpsimd.dma_start(wc[:], wc_d[:].rearrange("(k p) c -> p k c", k=4))
            pwr = cp.tile([P, COLS], f32)
            nc.gpsimd.dma_start(pwr[:], pw_d[:])
            rir = cp.tile([P, NR], f32)
            nc.gpsimd.dma_start(rir[:], ri_d[:])
            kir = cp.tile([P, NOPS], f32)
            nc.gpsimd.dma_start(kir[:], ki_d[:])
            tgr = cp.tile([P, T], f32)
            nc.gpsimd.dma_start(tgr[:], tg_d[:])
            ident = cp.tile([P, P], f32)
            nc.gpsimd.dma_start(ident[:], id_d[:])
            w2tb = cp.tile([NR + 1, HID], f32)
            nc.gpsimd.dma_start(w2tb[:], w2_d[:])
            lngr = cp.tile([P, HID], f32)
            nc.gpsimd.dma_start(lngr[:], lg_d[:])
            lnbr = cp.tile([P, HID], f32)
            nc.gpsimd.dma_start(lnbr[:], lb_d[:])
            bt = cp.tile([P, G + 1], f32)
            for i in range(G):
                nc.vector.memset(bt[:, i:i + 1], float(delta[i]))
            nc.vector.memset(bt[:, G:G + 1], 1e-5)

            pwb = pwr[:]                                          # [P,COLS]
            lngb = lngr[:]                                        # [P,HID]
            lnbb = lnbr[:]                                        # [P,HID]
            rib = rir[:].unsqueeze(1).broadcast_to([P, 3 * G * T, NR])
            kib = (kir[:].unsqueeze(1).unsqueeze(1)               # [P,1,1,NOPS]
                   .broadcast_to([P, G, T, NOPS]))
            tgb = tgr[:].unsqueeze(1).broadcast_to([P, G, T])

            for blk in range(NBLK):
                r0, r1 = blk * P, (blk + 1) * P
                # ---- stage A: logits = z_blk @ W_cat ----
                zb = wp.tile([P, HID], f32)
                nc.gpsimd.dma_start(zb[:], z_d[r0:r1, :])
                zt = wp.tile([P, 4, P], f32)
                for k in range(4):
                    tp = pt.tile([P, P], f32)
                    nc.tensor.transpose(tp[:], zb[:, k * P:(k + 1) * P], ident[:])
                    nc.scalar.activation(zt[:, k, :], tp[:], AF.Copy)
                l1 = pl.tile([P, 512], f32)
                l2 = pl.tile([P, COLS - 512], f32)
                for k in range(4):
                    nc.tensor.matmul(l1[:], zt[:, k, :], wc[:, k, 0:512],
                                     start=(k == 0), stop=(k == 3))
                for k in range(4):
                    nc.tensor.matmul(l2[:], zt[:, k, :], wc[:, k, 512:COLS],
                                     start=(k == 0), stop=(k == 3))
                lg = wp.tile([P, COLS], f32)
                nc.scalar.activation(lg[:, 0:512], l1[:], AF.Copy)
                nc.scalar.activation(lg[:, 512:COLS], l2[:], AF.Copy)

                # ---- per-candidate sigmoid decode -> decimals ----
                dvals = wp.tile([P, 3, G, T], f32)   # a-order [s1,s2,dst]
                opd = wp.tile([P, G, T], f32)
                plen = wp.tile([P, G], f32)
                S = wp.tile([P, 2, G, NR], f32)      # [R;M] state
                for g in range(G):
                    sig = sp.tile([P, COLS], f32)
                    nc.scalar.activation(sig[:], lg[:], AF.Sigmoid,
                                         bias=bt[:, g:g + 1])
                    nc.vector.tensor_tensor(sig[:], sig[:], pwb, OP.mult)
                    nc.vector.tensor_reduce(
                        S[:, 0, g, :],
                        sig[:, OFF_R:OFF_OP].rearrange("p (r b) -> p r b", r=NR),
                        AX.X, OP.add)
                    nc.vector.tensor_reduce(
                        opd[:, g, :],
                        sig[:, OFF_OP:OFF_D].rearrange("p (t b) -> p t b", t=T),
                        AX.X, OP.add)
                    nc.vector.tensor_reduce(
                        dvals[:, 2, g, :],
                        sig[:, OFF_D:OFF_1].rearrange("p (t b) -> p t b", t=T),
                        AX.X, OP.add)
                    nc.vector.tensor_reduce(
                        dvals[:, 0, g, :],
                        sig[:, OFF_1:OFF_2].rearrange("p (t b) -> p t b", t=T),
                        AX.X, OP.add)
                    nc.vector.tensor_reduce(
                        dvals[:, 1, g, :],
                        sig[:, OFF_2:OFF_L].rearrange("p (t b) -> p t b", t=T),
                        AX.X, OP.add)
                    nc.vector.tensor_reduce(plen[:, g:g + 1],
                                            sig[:, OFF_L:COLS], AX.X, OP.add)

                # ---- soft halting mask ----
                actx = wp.tile([P, G, T], f32)
                nc.vector.tensor_tensor(
                    actx[:], plen[:].unsqueeze(2).broadcast_to([P, G, T]),
                    tgb, OP.subtract)
                nc.scalar.activation(actx[:], actx[:], AF.Sigmoid)

                # ---- softmax numerators over registers / memory addrs ----
                nb = wp.tile([P, 3, G, T, NR], f32)
                nc.vector.tensor_tensor(
                    nb[:].rearrange("p a g t r -> p (a g t) r"),
                    dvals[:].rearrange("p a g t -> p (a g t)")
                    .unsqueeze(2).broadcast_to([P, 3 * G * T, NR]),
                    rib, OP.subtract)
                nbf = nb[:].rearrange("p a g t r -> p (a g t r)")
                nc.scalar.activation(nbf, nbf, AF.Square)
                nc.scalar.activation(nbf, nbf, AF.Exp, scale=-1.0)

                ob = wp.tile([P, G, T, NOPS], f32)
                nc.vector.tensor_tensor(
                    ob[:], opd[:].unsqueeze(3).broadcast_to([P, G, T, NOPS]),
                    kib, OP.subtract)
                obf = ob[:].rearrange("p g t k -> p (g t k)")
                nc.scalar.activation(obf, obf, AF.Square)
                nc.scalar.activation(obf, obf, AF.Exp, scale=-1.0)

                # ---- partition functions + reciprocals ----
                Zb = wp.tile([P, 3, G, T], f32)
                iZ = wp.tile([P, 3, G, T], f32)
                nc.vector.tensor_reduce(
                    Zb[:].rearrange("p a g t -> p (a g t)"),
                    nb[:].rearrange("p a g t r -> p (a g t) r"), AX.X, OP.add)
                nc.vector.reciprocal(iZ[:], Zb[:])
                Zop = wp.tile([P, G, T], f32)
                iZop = wp.tile([P, G, T], f32)
                nc.vector.tensor_reduce(Zop[:], ob[:], AX.X, OP.add)
                nc.vector.reciprocal(iZop[:], Zop[:])

                # ---- fold softmax denominators into per-step coefficients ----
                # coefT kinds [A,C,B,D] pair with vbuf kinds [v1n,lvn,v2n,dvn]
                coefT = wp.tile([P, T, 4, G], f32)
                cRM = wp.tile([P, T, 2, G], f32)
                iZ1 = iZ[:, 0, :, :]
                iZ2 = iZ[:, 1, :, :]
                iZd = iZ[:, 2, :, :]
                slotA = coefT[:, :, 0, :].transpose([0, 2, 1])
                slotC = coefT[:, :, 1, :].transpose([0, 2, 1])
                slotB = coefT[:, :, 2, :].transpose([0, 2, 1])
                slotD = coefT[:, :, 3, :].transpose([0, 2, 1])
                slot_cR = cRM[:, :, 0, :].transpose([0, 2, 1])
                slot_cM = cRM[:, :, 1, :].transpose([0, 2, 1])
                t1 = sp.tile([P, G, T], f32)
                t2 = sp.tile([P, G, T], f32)
                nc.vector.tensor_tensor(t1[:], ob[:, :, :, 0], ob[:, :, :, 1], OP.add)
                nc.vector.tensor_tensor(t1[:], t1[:], iZop[:], OP.mult)
                nc.vector.tensor_tensor(slotA, t1[:], iZ1, OP.mult)
                nc.vector.tensor_tensor(t1[:], ob[:, :, :, 0], ob[:, :, :, 1], OP.subtract)
                nc.vector.tensor_tensor(t1[:], t1[:], iZop[:], OP.mult)
                nc.vector.tensor_tensor(slotB, t1[:], iZ2, OP.mult)
                nc.vector.tensor_tensor(t1[:], ob[:, :, :, 2], iZop[:], OP.mult)
                nc.vector.tensor_tensor(slotC, t1[:], iZ1, OP.mult)
                nc.vector.tensor_tensor(t2[:], ob[:, :, :, 3], iZop[:], OP.mult)
                nc.vector.tensor_tensor(slotD, t2[:], iZd, OP.mult)
                nc.vector.tensor_tensor(t1[:], t2[:], iZd, OP.mult)
                nc.vector.tensor_tensor(slot_cM, t1[:], actx[:], OP.mult)
                nc.vector.tensor_scalar(t2[:], t2[:], -1.0, 1.0, OP.mult, OP.add)
                nc.vector.tensor_tensor(t2[:], t2[:], iZd, OP.mult)
                nc.vector.tensor_tensor(slot_cR, t2[:], actx[:], OP.mult)

                # ---- soft interpreter scan over T steps ----
                nc.vector.memset(S[:, 1, :, :], 0.0)
                vbuf = wp.tile([P, 4, G], f32)    # [v1n, lvn, v2n, dvn]
                targ = wp.tile([P, 2, G], f32)    # [res, v1]
                Pq = wp.tile([P, 2, G, NR], f32)
                GD = wp.tile([P, 2, G, NR], f32)
                resP = wp.tile([P, 4, G], f32)
                for t in range(T):
                    n1t = nb[:, 0, :, t, :]
                    nc.vector.tensor_tensor(
                        Pq[:], S[:],
                        n1t.unsqueeze(1).broadcast_to([P, 2, G, NR]), OP.mult)
                    nc.vector.tensor_reduce(vbuf[:, 0:2, :], Pq[:], AX.X, OP.add)
                    nc.vector.tensor_tensor(
                        Pq[:],
                        S[:, 0, :, :].unsqueeze(1).broadcast_to([P, 2, G, NR]),
                        nb[:, 1:3, :, t, :], OP.mult)
                    nc.vector.tensor_reduce(vbuf[:, 2:4, :], Pq[:], AX.X, OP.add)
                    nc.vector.tensor_tensor(resP[:], vbuf[:], coefT[:, t, :, :],
                                            OP.mult)
                    nc.vector.tensor_reduce(targ[:, 0, :],
                                            resP[:].transpose([0, 2, 1]),
                                            AX.X, OP.add)
                    nc.vector.tensor_tensor(targ[:, 1, :], vbuf[:, 0, :],
                                            iZ[:, 0, :, t], OP.mult)
                    nc.vector.tensor_tensor(
                        Pq[:], S[:],
                        targ[:].unsqueeze(3).broadcast_to([P, 2, G, NR]),
                        OP.subtract)
                    nc.vector.tensor_tensor(
                        GD[:],
                        cRM[:, t, :, :].unsqueeze(3).broadcast_to([P, 2, G, NR]),
                        nb[:, 2, :, t, :].unsqueeze(1).broadcast_to([P, 2, G, NR]),
                        OP.mult)
                    nc.vector.tensor_tensor(GD[:], GD[:], Pq[:], OP.mult)
                    nc.vector.tensor_tensor(S[:], S[:], GD[:], OP.subtract)

                # ---- register2hidden + LayerNorm, per candidate ----
                for g in range(G):
                    rp = ph.tile([NR, P], f32)
                    nc.tensor.transpose(rp[:], S[:, 0, g, :], ident[:])
                    rft = sp.tile([NR + 1, P], f32)
                    nc.scalar.activation(rft[0:NR, :], rp[:], AF.Copy)
                    nc.vector.memset(rft[NR:NR + 1, :], 1.0)
                    hp = ph.tile([P, HID], f32)
                    nc.tensor.matmul(hp[:], rft[:], w2tb[:], start=True, stop=True)
                    h = sp.tile([P, HID], f32)
                    hsum = sp.tile([P, 1], f32)
                    nc.scalar.activation(h[:], hp[:], AF.Copy, accum_out=hsum[:])
                    negmu = sp.tile([P, 1], f32)
                    nc.vector.tensor_scalar_mul(negmu[:], hsum[:], -1.0 / HID)
                    hc = sp.tile([P, HID], f32)
                    nc.vector.tensor_scalar_add(hc[:], h[:], negmu[:])
                    sq = sp.tile([P, HID], f32)
                    vsum = sp.tile([P, 1], f32)
                    nc.scalar.activation(sq[:], hc[:], AF.Square,
                                         accum_out=vsum[:])
                    std = sp.tile([P, 1], f32)
                    rstd = sp.tile([P, 1], f32)
                    nc.scalar.activation(std[:], vsum[:], AF.Sqrt,
                                         bias=bt[:, G:G + 1], scale=1.0 / HID)
                    nc.vector.reciprocal(rstd[:], std[:])
                    ot = sp.tile([P, HID], f32)
                    nc.vector.scalar_tensor_tensor(ot[:], hc[:], rstd[:], lngb,
                                                   OP.mult, OP.mult)
                    nc.vector.tensor_tensor(ot[:], ot[:], lnbb, OP.add)
                    nc.gpsimd.dma_start(
                        out_d[r0:r1, g * HID:(g + 1) * HID], ot[:])

    nc.compile()
    return nc


def _get_nc():
    if "nc" not in _STATE:
        _STATE["nc"] = _build()
    return _STATE["nc"]


def _make_consts(inputs):
    f = lambda a: np.ascontiguousarray(np.asarray(a), dtype=np.float32)
    wcat = np.concatenate([f(inputs["W_R"]), f(inputs["W_op"]),
                           f(inputs["W_dst"]), f(inputs["W_src1"]),
                           f(inputs["W_src2"]), f(inputs["W_len"])], axis=1)
    pw8 = (2.0 ** np.arange(NB)).astype(np.float32)
    pw2 = (2.0 ** np.arange(OPB)).astype(np.float32)
    pw5 = (2.0 ** np.arange(AB)).astype(np.float32)
    pw = np.concatenate([np.tile(pw8, NR), np.tile(pw2, T),
                         np.tile(pw5, T), np.tile(pw5, T), np.tile(pw5, T),
                         pw5]).astype(np.float32)
    w2tb = np.vstack([f(inputs["W_r2h"]).T, f(inputs["b_r2h"])[None]])
    rep = lambda row: np.ascontiguousarray(np.tile(row[None], (P, 1)))
    return {
        "wcat": np.ascontiguousarray(wcat),
        "pw": rep(pw),
        "ri": rep(np.arange(NR, dtype=np.float32)),
        "ki": rep(np.arange(NOPS, dtype=np.float32)),
        "tg": rep(np.arange(T, dtype=np.float32) + 0.5),
        "ident": np.eye(P, dtype=np.float32),
        "w2tb": np.ascontiguousarray(w2tb),
        "lng": rep(f(inputs["ln_g"])),
        "lnb": rep(f(inputs["ln_b"])),
    }


def kernel(**inputs) -> np.ndarray:
    nc = _get_nc()
    z = np.ascontiguousarray(np.asarray(inputs["z_hidden"]), dtype=np.float32)
    consts = _make_consts(inputs)
    in_maps = [dict(z=np.ascontiguousarray(z[c * BC:(c + 1) * BC]), **consts)
               for c in range(NCORES)]
    res = run_bass_kernel_spmd(nc, in_maps, list(range(NCORES)))
    out = np.concatenate(
        [np.asarray(res.results[c]["out"]) for c in range(NCORES)], axis=0)
    return out.reshape(B, G, HID)

